# revision 6
# baseline (speedup 1.0000x reference)
"""Trainium2 Bass kernel for a 2-layer GAT node-pair scorer.

Sharding: edges are partitioned by destination node (dst-sorted) across 8
cores; each core owns a contiguous range of nodes and all their incoming
edges, so the per-node softmax and aggregation are core-local. The per-node
feature tables (g = h@W plus attention logits) are computed replicated on
every core; after each GAT layer only the new node features (1.6 MB/core)
are AllGathered. Node pairs for the scorer are sharded across cores.
"""

import numpy as np

import concourse.bass as bass
import concourse.bacc as bacc
import concourse.mybir as mybir
import concourse.tile as tile
from concourse.bass_utils import run_bass_kernel_spmd

P = 128

F32 = mybir.dt.float32
I32 = mybir.dt.int32


class Cfg:
    def __init__(self, n_nodes, n_pairs, t_blk, n_cores=8,
                 in_dim=32, hid=64, heads=4):
        assert n_nodes % n_cores == 0
        self.C = n_cores
        self.N = n_nodes
        self.NS = n_nodes // n_cores          # nodes per core
        self.B = (self.NS + P - 1) // P        # node blocks per core
        self.WLAST = self.NS - (self.B - 1) * P
        self.T = t_blk                         # edge tiles per node block
        self.IN = in_dim
        self.HID = hid
        self.H = heads
        self.G = heads * hid                   # 256
        self.GA = self.G + heads               # g row + a_src = 260
        self.GAD = self.GA + heads             # + a_dst = 264
        self.PAIRS = n_pairs
        assert n_pairs % n_cores == 0
        self.PS = n_pairs // n_cores
        self.PT = (self.PS + P - 1) // P


def block_width(cfg, b):
    return cfg.WLAST if b == cfg.B - 1 else P


def build_program(cfg: Cfg):
    nc = bacc.Bacc("TRN2", target_bir_lowering=False, num_devices=cfg.C)
    C, B, T, NS, N = cfg.C, cfg.B, cfg.T, cfg.NS, cfg.N
    HID, IN, H, G, GA, GAD = cfg.HID, cfg.IN, cfg.H, cfg.G, cfg.GA, cfg.GAD
    PT = cfg.PT
    ET = B * T                                  # edge-tile columns per core

    # ---------------- external inputs ----------------
    xT = nc.dram_tensor("xT", [IN, NS], F32, kind="ExternalInput")
    w_in = nc.dram_tensor("w_in", [IN, HID], F32, kind="ExternalInput")
    wc0 = nc.dram_tensor("wc0", [HID, GAD], F32, kind="ExternalInput")
    wc1 = nc.dram_tensor("wc1", [HID, GAD], F32, kind="ExternalInput")
    w1ab = nc.dram_tensor("w1ab", [HID, 2 * HID], F32, kind="ExternalInput")
    bin_r = nc.dram_tensor("bin_r", [P, HID], F32, kind="ExternalInput")
    bg0_r = nc.dram_tensor("bg0_r", [P, HID], F32, kind="ExternalInput")
    bg1_r = nc.dram_tensor("bg1_r", [P, HID], F32, kind="ExternalInput")
    bs1x_r = nc.dram_tensor("bs1x_r", [P, 2 * HID], F32, kind="ExternalInput")
    ws2_r = nc.dram_tensor("ws2_r", [P, HID], F32, kind="ExternalInput")
    colio = nc.dram_tensor("colio", [P, P], F32, kind="ExternalInput")
    ident = nc.dram_tensor("ident", [P, P], F32, kind="ExternalInput")
    esrc = nc.dram_tensor("esrc", [P, ET], I32, kind="ExternalInput")
    edst = nc.dram_tensor("edst", [P, ET], I32, kind="ExternalInput")
    edstl = nc.dram_tensor("edstl", [P, ET], F32, kind="ExternalInput")
    pi_t = nc.dram_tensor("pi_t", [P, PT], I32, kind="ExternalInput")
    pj_t = nc.dram_tensor("pj_t", [P, PT], I32, kind="ExternalInput")

    scores_pad = nc.dram_tensor("scores_pad", [PT, P], F32, kind="ExternalOutput")

    # ---------------- internal DRAM ----------------
    gtab = [nc.dram_tensor(f"gtab{l}", [N, GA], F32) for l in range(2)]
    adtab = [nc.dram_tensor(f"adtab{l}", [N, H], F32) for l in range(2)]
    hT_shard = [nc.dram_tensor(f"hT_shard{l}", [HID, NS], F32) for l in range(2)]
    hT_full = [
        nc.dram_tensor(f"hT_full{l}", [HID * C, NS], F32, addr_space="Shared")
        for l in range(2)
    ]
    u_shard = nc.dram_tensor("u_shard", [NS, HID], F32)
    v_shard = nc.dram_tensor("v_shard", [NS, HID], F32)
    u_full = nc.dram_tensor("u_full", [N, HID], F32, addr_space="Shared")
    v_full = nc.dram_tensor("v_full", [N, HID], F32, addr_space="Shared")

    groups = [list(range(C))]

    with tile.TileContext(nc) as tc:
        with (
            tc.tile_pool(name="cst", bufs=1) as cst,
            tc.tile_pool(name="work", bufs=6) as wk,
            tc.tile_pool(name="oh", bufs=4) as ohp,
            tc.tile_pool(name="rhs", bufs=4) as rhp,
            tc.tile_pool(name="small", bufs=6) as sm,
            tc.tile_pool(name="epi", bufs=3) as ep,
            tc.tile_pool(name="ps_mm", bufs=2, space="PSUM") as ps_mm,
            tc.tile_pool(name="ps_agg", bufs=2, space="PSUM") as ps_agg,
            tc.tile_pool(name="ps_t", bufs=2, space="PSUM") as ps_t,
        ):
            # ---------- constants to SBUF ----------
            xT_sb = cst.tile([IN, NS], F32)
            nc.sync.dma_start(xT_sb[:], xT[:])
            w_in_sb = cst.tile([IN, HID], F32)
            nc.sync.dma_start(w_in_sb[:], w_in[:])
            wc_sb = [cst.tile([HID, GAD], F32, name=f"wc_sb{l}", tag=f"wc{l}")
                     for l in range(2)]
            nc.sync.dma_start(wc_sb[0][:], wc0[:])
            nc.sync.dma_start(wc_sb[1][:], wc1[:])
            w1ab_sb = cst.tile([HID, 2 * HID], F32)
            nc.sync.dma_start(w1ab_sb[:], w1ab[:])
            bin_sb = cst.tile([P, HID], F32)
            nc.sync.dma_start(bin_sb[:], bin_r[:])
            bg_sb = [cst.tile([P, HID], F32, name=f"bg_sb{l}", tag=f"bg{l}")
                     for l in range(2)]
            nc.sync.dma_start(bg_sb[0][:], bg0_r[:])
            nc.sync.dma_start(bg_sb[1][:], bg1_r[:])
            bs1x_sb = cst.tile([P, 2 * HID], F32)
            nc.sync.dma_start(bs1x_sb[:], bs1x_r[:])
            ws2_sb = cst.tile([P, HID], F32)
            nc.sync.dma_start(ws2_sb[:], ws2_r[:])
            colio_sb = cst.tile([P, P], F32)
            nc.sync.dma_start(colio_sb[:], colio[:])
            ident_sb = cst.tile([P, P], F32)
            nc.sync.dma_start(ident_sb[:], ident[:])
            esrc_sb = cst.tile([P, ET], I32)
            nc.sync.dma_start(esrc_sb[:], esrc[:])
            edst_sb = cst.tile([P, ET], I32)
            nc.sync.dma_start(edst_sb[:], edst[:])
            edstl_sb = cst.tile([P, ET], F32)
            nc.sync.dma_start(edstl_sb[:], edstl[:])
            pi_sb = cst.tile([P, PT], I32)
            nc.sync.dma_start(pi_sb[:], pi_t[:])
            pj_sb = cst.tile([P, PT], I32)
            nc.sync.dma_start(pj_sb[:], pj_t[:])
            scores_sb = cst.tile([P, PT], F32)

            tc.strict_bb_all_engine_barrier()

            # ---------- phase B: h0 = lrelu(x @ W_in + b_in), store transposed ----------
            for b in range(B):
                w = block_width(cfg, b)
                ps_h = ps_mm.tile([P, GAD], F32, tag="mm")
                nc.tensor.matmul(ps_h[:w, :HID], lhsT=xT_sb[:, b * P:b * P + w],
                                 rhs=w_in_sb[:], start=True, stop=True)
                hb = ep.tile([P, HID], F32, tag="hb")
                nc.vector.tensor_add(out=hb[:w], in0=ps_h[:w, :HID], in1=bin_sb[:w])
                hn = ep.tile([P, HID], F32, tag="hn")
                nc.vector.scalar_tensor_tensor(
                    out=hn[:w], in0=hb[:w], scalar=0.01, in1=hb[:w],
                    op0=mybir.AluOpType.mult, op1=mybir.AluOpType.max)
                psT = ps_t.tile([P, P], F32, tag="pst")
                nc.tensor.transpose(psT[:HID, :w], hn[:w], ident_sb[:w, :w])
                hT_sb = ep.tile([HID, P], F32, tag="hT")
                nc.vector.tensor_copy(out=hT_sb[:, :w], in_=psT[:HID, :w])
                nc.sync.dma_start(out=hT_shard[0][:, b * P:b * P + w],
                                  in_=hT_sb[:, :w])

            # ---------- two GAT layers ----------
            for l in range(2):
                nc.gpsimd.collective_compute(
                    "AllGather", mybir.AluOpType.bypass, replica_groups=groups,
                    ins=[hT_shard[l][:]], outs=[hT_full[l][:]])

                # g-phase: tables for all N nodes (replicated on every core)
                for nb in range(C * B):
                    r, b = divmod(nb, B)
                    w = block_width(cfg, b)
                    base = r * NS + b * P
                    lhsT = wk.tile([HID, P], F32, tag="glhs")
                    nc.sync.dma_start(
                        out=lhsT[:, :w],
                        in_=hT_full[l][r * HID:(r + 1) * HID, b * P:b * P + w])
                    ps_g = ps_mm.tile([P, GAD], F32, tag="mm")
                    nc.tensor.matmul(ps_g[:w], lhsT=lhsT[:, :w], rhs=wc_sb[l][:],
                                     start=True, stop=True)
                    g_sb = wk.tile([P, GA], F32, tag="g_sb")
                    nc.vector.tensor_copy(out=g_sb[:w], in_=ps_g[:w, :GA])
                    ad_sb = sm.tile([P, H], F32, tag="ad_sb")
                    nc.vector.tensor_copy(out=ad_sb[:w], in_=ps_g[:w, GA:GAD])
                    nc.sync.dma_start(out=gtab[l][base:base + w, :], in_=g_sb[:w])
                    nc.sync.dma_start(out=adtab[l][base:base + w, :], in_=ad_sb[:w])

                # edge phase: per-block softmax-weighted aggregation
                for b in range(B):
                    w = block_width(cfg, b)
                    ps_o = ps_agg.tile([P, GA], F32, tag="ps_o")
                    for t in range(T):
                        col = b * T + t
                        g_e = wk.tile([P, GA], F32, tag="g_e")
                        nc.gpsimd.indirect_dma_start(
                            out=g_e[:], out_offset=None, in_=gtab[l][:],
                            in_offset=bass.IndirectOffsetOnAxis(
                                ap=esrc_sb[:, col:col + 1], axis=0))
                        ad_e = sm.tile([P, H], F32, tag="ad_e")
                        nc.gpsimd.indirect_dma_start(
                            out=ad_e[:], out_offset=None, in_=adtab[l][:],
                            in_offset=bass.IndirectOffsetOnAxis(
                                ap=edst_sb[:, col:col + 1], axis=0))
                        asum = sm.tile([P, H], F32, tag="asum")
                        nc.vector.tensor_add(out=asum[:], in0=g_e[:, G:GA],
                                             in1=ad_e[:])
                        lr = sm.tile([P, H], F32, tag="lr")
                        nc.vector.scalar_tensor_tensor(
                            out=lr[:], in0=asum[:], scalar=0.2, in1=asum[:],
                            op0=mybir.AluOpType.mult, op1=mybir.AluOpType.max)
                        rhs = rhp.tile([P, GA], F32, tag="rhs")
                        nc.scalar.activation(out=rhs[:, G:GA], in_=lr[:],
                                             func=mybir.ActivationFunctionType.Exp)
                        nc.vector.tensor_tensor(
                            out=rhs[:, 0:G].rearrange("p (h o) -> p h o", h=H),
                            in0=g_e[:, 0:G].rearrange("p (h o) -> p h o", h=H),
                            in1=rhs[:, G:GA].to_broadcast([P, H, HID]),
                            op=mybir.AluOpType.mult)
                        oh = ohp.tile([P, P], F32, tag="oh")
                        nc.vector.tensor_tensor(
                            out=oh[:],
                            in0=edstl_sb[:, col:col + 1].to_broadcast([P, P]),
                            in1=colio_sb[:], op=mybir.AluOpType.is_equal)
                        nc.tensor.matmul(ps_o[:], lhsT=oh[:], rhs=rhs[:],
                                         start=(t == 0), stop=(t == T - 1))

                    # block epilogue: h_new = mean_h(num/den) + bias (+lrelu)
                    den = sm.tile([P, H], F32, tag="den")
                    nc.scalar.mul(den[:w], ps_o[:w, G:GA], float(H))
                    rec = sm.tile([P, H], F32, tag="rec")
                    nc.vector.reciprocal(out=rec[:w], in_=den[:w])
                    tmp = ep.tile([P, G], F32, tag="tmp")
                    nc.vector.tensor_tensor(
                        out=tmp[:w].rearrange("p (h o) -> p h o", h=H),
                        in0=ps_o[:w, 0:G].rearrange("p (h o) -> p h o", h=H),
                        in1=rec[:w].to_broadcast([w, H, HID]),
                        op=mybir.AluOpType.mult)
                    hs = ep.tile([P, HID], F32, tag="hs")
                    nc.vector.tensor_reduce(
                        out=hs[:w], in_=tmp[:w].rearrange("p (h o) -> p o h", h=H),
                        axis=mybir.AxisListType.X, op=mybir.AluOpType.add)
                    hb2 = ep.tile([P, HID], F32, tag="hb2")
                    nc.vector.tensor_add(out=hb2[:w], in0=hs[:w], in1=bg_sb[l][:w])
                    if l == 0:
                        hn2 = ep.tile([P, HID], F32, tag="hn2")
                        nc.vector.scalar_tensor_tensor(
                            out=hn2[:w], in0=hb2[:w], scalar=0.01, in1=hb2[:w],
                            op0=mybir.AluOpType.mult, op1=mybir.AluOpType.max)
                    else:
                        hn2 = hb2
                    psT2 = ps_t.tile([P, P], F32, tag="pst")
                    nc.tensor.transpose(psT2[:HID, :w], hn2[:w], ident_sb[:w, :w])
                    hT2_sb = ep.tile([HID, P], F32, tag="hT2")
                    nc.vector.tensor_copy(out=hT2_sb[:, :w], in_=psT2[:HID, :w])
                    if l == 0:
                        nc.sync.dma_start(out=hT_shard[1][:, b * P:b * P + w],
                                          in_=hT2_sb[:, :w])
                    else:
                        ps_uv = ps_mm.tile([P, GAD], F32, tag="mm")
                        nc.tensor.matmul(ps_uv[:w, :2 * HID], lhsT=hT2_sb[:, :w],
                                         rhs=w1ab_sb[:], start=True, stop=True)
                        uv = ep.tile([P, 2 * HID], F32, tag="uv")
                        nc.vector.tensor_add(out=uv[:w], in0=ps_uv[:w, :2 * HID],
                                             in1=bs1x_sb[:w])
                        nc.sync.dma_start(out=u_shard[b * P:b * P + w, :],
                                          in_=uv[:w, 0:HID])
                        nc.sync.dma_start(out=v_shard[b * P:b * P + w, :],
                                          in_=uv[:w, HID:2 * HID])

            # ---------- scorer ----------
            nc.gpsimd.collective_compute(
                "AllGather", mybir.AluOpType.bypass, replica_groups=groups,
                ins=[u_shard[:]], outs=[u_full[:]])
            nc.gpsimd.collective_compute(
                "AllGather", mybir.AluOpType.bypass, replica_groups=groups,
                ins=[v_shard[:]], outs=[v_full[:]])

            for pt in range(PT):
                ui = wk.tile([P, HID], F32, tag="ui")
                nc.gpsimd.indirect_dma_start(
                    out=ui[:], out_offset=None, in_=u_full[:],
                    in_offset=bass.IndirectOffsetOnAxis(
                        ap=pi_sb[:, pt:pt + 1], axis=0))
                vj = wk.tile([P, HID], F32, tag="vj")
                nc.gpsimd.indirect_dma_start(
                    out=vj[:], out_offset=None, in_=v_full[:],
                    in_offset=bass.IndirectOffsetOnAxis(
                        ap=pj_sb[:, pt:pt + 1], axis=0))
                s0 = sm.tile([P, HID], F32, tag="s0")
                nc.vector.tensor_add(out=s0[:], in0=ui[:], in1=vj[:])
                s1 = sm.tile([P, HID], F32, tag="s1")
                nc.vector.scalar_tensor_tensor(
                    out=s1[:], in0=s0[:], scalar=0.01, in1=s0[:],
                    op0=mybir.AluOpType.mult, op1=mybir.AluOpType.max)
                s2 = sm.tile([P, HID], F32, tag="s2")
                nc.vector.tensor_tensor(out=s2[:], in0=s1[:], in1=ws2_sb[:],
                                        op=mybir.AluOpType.mult)
                nc.vector.tensor_reduce(
                    out=scores_sb[:, pt:pt + 1], in_=s2[:],
                    axis=mybir.AxisListType.X, op=mybir.AluOpType.add)

            ps_sc = ps_t.tile([P, P], F32, tag="pst")
            nc.tensor.transpose(ps_sc[:PT, :], scores_sb[:, :PT], ident_sb[:])
            sc_T = ep.tile([PT, P], F32, tag="sc_T")
            nc.vector.tensor_copy(out=sc_T[:], in_=ps_sc[:PT, :])
            nc.sync.dma_start(out=scores_pad[:], in_=sc_T[:])

    nc.compile()
    return nc


def host_prep(inputs, n_cores=8):
    x = np.asarray(inputs["x"], np.float32)
    ei = np.asarray(inputs["edge_index"], np.int64)
    node_i = np.asarray(inputs["node_i"], np.int64)
    node_j = np.asarray(inputs["node_j"], np.int64)
    N, IN = x.shape
    HID = np.asarray(inputs["W_in"]).shape[1]
    H = np.asarray(inputs["att0"]).shape[0]

    loop = np.arange(N, dtype=np.int64)
    src = np.concatenate([ei[0], loop])
    dst = np.concatenate([ei[1], loop])
    order = np.argsort(dst, kind="stable")
    src_s = src[order].astype(np.int32)
    dst_s = dst[order].astype(np.int32)

    NS = N // n_cores
    B = (NS + P - 1) // P
    # per-(core,block) edge counts -> uniform tile count
    blk_of_edge = dst_s // P if NS % P == 0 else None
    gblk = (dst_s // NS) * B + (dst_s % NS) // P
    cnt = np.bincount(gblk, minlength=n_cores * B)
    t_blk = int(np.ceil(cnt.max() / P))
    cfg = Cfg(N, len(node_i), t_blk, n_cores=n_cores, in_dim=IN, hid=HID, heads=H)

    # weight combos
    Wg = [np.asarray(inputs["Wg0"], np.float32), np.asarray(inputs["Wg1"], np.float32)]
    att = [np.asarray(inputs["att0"], np.float32), np.asarray(inputs["att1"], np.float32)]
    wcs = []
    for l in range(2):
        A_src = np.zeros((H * HID, H), np.float32)
        A_dst = np.zeros((H * HID, H), np.float32)
        for h in range(H):
            A_dst[h * HID:(h + 1) * HID, h] = att[l][h, :HID]
            A_src[h * HID:(h + 1) * HID, h] = att[l][h, HID:]
        wcs.append(np.ascontiguousarray(
            np.hstack([Wg[l], Wg[l] @ A_src, Wg[l] @ A_dst]), dtype=np.float32))

    Ws1 = np.asarray(inputs["Ws1"], np.float32)
    w1ab = np.ascontiguousarray(np.hstack([Ws1[:HID], Ws1[HID:]]), np.float32)
    bs1 = np.asarray(inputs["bs1"], np.float32)
    bs1x = np.concatenate([bs1, np.zeros(HID, np.float32)])
    ws2 = np.asarray(inputs["Ws2"], np.float32).reshape(-1)

    def rep(v, width=None):
        v = np.asarray(v, np.float32).reshape(1, -1)
        return np.ascontiguousarray(np.tile(v, (P, 1)))

    common = {
        "w_in": np.ascontiguousarray(np.asarray(inputs["W_in"], np.float32)),
        "wc0": wcs[0], "wc1": wcs[1], "w1ab": w1ab,
        "bin_r": rep(inputs["b_in"]), "bg0_r": rep(inputs["bg0"]),
        "bg1_r": rep(inputs["bg1"]), "bs1x_r": rep(bs1x), "ws2_r": rep(ws2),
        "colio": np.ascontiguousarray(
            np.tile(np.arange(P, dtype=np.float32)[None, :], (P, 1))),
        "ident": np.eye(P, dtype=np.float32),
    }

    ET = B * cfg.T
    in_maps = []
    for c in range(n_cores):
        lo = np.searchsorted(dst_s, c * NS)
        hi = np.searchsorted(dst_s, (c + 1) * NS)
        s_c = src_s[lo:hi]
        d_c = dst_s[lo:hi]
        d_loc = d_c - c * NS
        blk = d_loc // P
        blk_starts = np.searchsorted(blk, np.arange(B))
        pos = np.arange(len(blk)) - blk_starts[blk]
        col = blk * cfg.T + pos // P
        lane = pos % P
        assert pos.max(initial=0) < cfg.T * P

        esrc = np.zeros((P, ET), np.int32)
        edst = np.zeros((P, ET), np.int32)
        edstl = np.full((P, ET), -1.0, np.float32)
        esrc[lane, col] = s_c
        edst[lane, col] = d_c
        edstl[lane, col] = (d_loc - blk * P).astype(np.float32)

        pi = np.zeros((P, cfg.PT), np.int32)
        pj = np.zeros((P, cfg.PT), np.int32)
        pr = np.arange(cfg.PS)
        pi[pr % P, pr // P] = node_i[c * cfg.PS:(c + 1) * cfg.PS].astype(np.int32)
        pj[pr % P, pr // P] = node_j[c * cfg.PS:(c + 1) * cfg.PS].astype(np.int32)

        xT_c = np.ascontiguousarray(x[c * NS:(c + 1) * NS].T)
        in_maps.append({**common, "xT": xT_c, "esrc": esrc, "edst": edst,
                        "edstl": edstl, "pi_t": pi, "pj_t": pj})
    return cfg, in_maps


_PROG_CACHE = {}


def run_full(inputs, trace=False):
    cfg, in_maps = host_prep(inputs)
    key = (cfg.N, cfg.T, cfg.PAIRS)
    if key not in _PROG_CACHE:
        _PROG_CACHE[key] = build_program(cfg)
    nc = _PROG_CACHE[key]
    res = run_bass_kernel_spmd(nc, in_maps, list(range(cfg.C)), trace=trace)
    outs = []
    for c in range(cfg.C):
        sp = res.results[c]["scores_pad"].reshape(-1)[:cfg.PS]
        outs.append(sp)
    scores = np.concatenate(outs).astype(np.float32)
    return scores, res


def kernel(**inputs):
    scores, _ = run_full(inputs)
    return scores
